# revision 6
# baseline (speedup 1.0000x reference)
"""Trainium2 kernel for nn_AnchorMlgDml (retrieval_knn).

Pipeline per core (data-parallel over N=65536 rows, 8 cores x 8192 rows):
  x_dml = mish(mish(x @ W1.T + b1) @ W2.T + b2)            [8192, 128]
  dist  = sqrt(x2 + a2 - 2 * x_dml @ anchors.T)            [8192, 1024]
  out   = log_softmax(dist @ Wc.T + bc)                    [8192, 10]

anchors = encode_dml(tanh(Wa @ mlg)) is x-independent: computed on the host
(float64) and replicated to all cores per the sharding hint.

The toolchain's ACT tables have no Mish/Softplus, so mish is evaluated as
  mish(x) = x * (tanh(a1*x + c1) + tanh(a2*x + c2) + 2) / 4
(fitted; end-to-end error ~1.6e-3, at the bf16 floor).  The /4 is folded
into host-side constants: W2.T/4, anchors.T/2 (for -2*x.a), ones/16 (x2).

On-device layout is transposed: features on partitions, rows on the free dim.
distT [anchors(part), rows(free)] feeds the logits GEMM directly (contraction
over anchors must be on partitions).  a2 rides the DVE scalar_tensor_tensor
per-partition scalar; x2 is replicated across partitions with a ones-matmul.

ACT table sets force a phase structure (switches cost ~2.7us):
  tanh phase -> sqrt phase -> exp/ln phase, enforced with add_dep_helper.
"""

import numpy as np
import ml_dtypes

import concourse.bass as bass
import concourse.bacc as bacc
import concourse.tile as tile
import concourse.mybir as mybir
from concourse.bass_utils import run_bass_kernel_spmd, checkenv
from concourse.tile_rust import add_dep_helper

N_CORES = 8
N, D, C = 65536, 128, 10
SS = 1024                    # anchors (sample_size)
RPC = N // N_CORES           # 8192 rows per core
CHUNK = 512
NCH = RPC // CHUNK           # 16 row chunks
NAT = SS // 128              # 8 anchor tiles

# mish(x) ~= x * (tanh(A1*x+C1) + tanh(A2*x+C2) + 2) / 4
A1, C1 = 0.94891, -0.15902
A2, C2 = 0.57573, 0.58034

BF = mybir.dt.bfloat16
F32 = mybir.dt.float32
F32R = mybir.dt.float32r
bf16 = ml_dtypes.bfloat16
AF = mybir.ActivationFunctionType
ALU = mybir.AluOpType

_CACHE = {}
last_result = None  # BassKernelResults of the most recent run (for test.py)


def _build():
    nc = bacc.Bacc("TRN2", target_bir_lowering=False, debug=False,
                   num_devices=N_CORES)

    xt_d = nc.dram_tensor("xt", [D, RPC], BF, kind="ExternalInput").ap()
    w1t_d = nc.dram_tensor("w1t", [D, D], BF, kind="ExternalInput").ap()
    w2t_d = nc.dram_tensor("w2t", [D, D], BF, kind="ExternalInput").ap()
    # per-layer ACT biases for the two tanh args (a_i*b + c_i) and the raw
    # layer bias (the multiplicand x = psum + b)
    bt_d = nc.dram_tensor("bt", [D, 4], F32, kind="ExternalInput").ap()
    bv_d = nc.dram_tensor("bv", [D, 2], F32, kind="ExternalInput").ap()
    m2at_d = nc.dram_tensor("m2at", [D, SS], BF, kind="ExternalInput").ap()
    a2_d = nc.dram_tensor("a2", [D, NAT], F32, kind="ExternalInput").ap()
    wct_d = nc.dram_tensor("wct", [D, NAT * C], BF, kind="ExternalInput").ap()
    bc_d = nc.dram_tensor("bc", [C, 1], F32, kind="ExternalInput").ap()
    ones_d = nc.dram_tensor("ones", [D, D], BF, kind="ExternalInput").ap()
    o10_d = nc.dram_tensor("o10", [C, 1], BF, kind="ExternalInput").ap()
    n10_d = nc.dram_tensor("n10", [1, C], F32, kind="ExternalInput").ap()
    out_d = nc.dram_tensor("out", [C, RPC], F32, kind="ExternalOutput").ap()

    with tile.TileContext(nc) as tc:
        with (
            tc.tile_pool(name="consts", bufs=1) as consts,
            tc.tile_pool(name="xin", bufs=1) as xin,
            tc.tile_pool(name="enc", bufs=3) as enc,
            tc.tile_pool(name="big", bufs=1) as big,
            tc.tile_pool(name="dwork", bufs=2) as dwork,
            tc.tile_pool(name="distp", bufs=2) as distp,
            tc.tile_pool(name="smx", bufs=3) as smx,
            tc.tile_pool(name="psE", bufs=2, space="PSUM") as psE,
            tc.tile_pool(name="psD", bufs=2, space="PSUM") as psD,
            tc.tile_pool(name="psX", bufs=2, space="PSUM") as psX,
            tc.tile_pool(name="psL", bufs=2, space="PSUM") as psL,
        ):
            # ---- constants (SWDGE ring; bulk x uses the HWDGE ring) ----
            w1t = consts.tile([D, D], BF)
            nc.gpsimd.dma_start(out=w1t, in_=w1t_d)
            w2t = consts.tile([D, D], BF)
            nc.gpsimd.dma_start(out=w2t, in_=w2t_d)
            bt = consts.tile([D, 4], F32)
            nc.gpsimd.dma_start(out=bt, in_=bt_d)
            bv = consts.tile([D, 2], F32)
            nc.gpsimd.dma_start(out=bv, in_=bv_d)
            m2at = consts.tile([D, SS], BF)
            nc.gpsimd.dma_start(out=m2at, in_=m2at_d)
            a2 = consts.tile([D, NAT], F32)
            nc.gpsimd.dma_start(out=a2, in_=a2_d)
            wct = consts.tile([D, NAT * C], BF)
            nc.gpsimd.dma_start(out=wct, in_=wct_d)
            bc = consts.tile([C, 1], F32)
            nc.gpsimd.dma_start(out=bc, in_=bc_d)
            ones = consts.tile([D, D], BF)
            nc.gpsimd.dma_start(out=ones, in_=ones_d)
            o10 = consts.tile([C, 1], BF)
            nc.gpsimd.dma_start(out=o10, in_=o10_d)
            n10 = consts.tile([1, C], F32)
            nc.gpsimd.dma_start(out=n10, in_=n10_d)

            # ---- x input: 4 quarter DMAs so compute starts early ----
            QW = RPC // 4
            xq = []
            for i in range(4):
                t = xin.tile([D, QW], BF, tag=f"xq{i}")
                nc.sync.dma_start(out=t, in_=xt_d[:, i * QW:(i + 1) * QW])
                xq.append(t)

            xdml = big.tile([D, RPC], BF)   # 4*mish(h2) (scale folded in consts)
            lsb = big.tile([C, RPC], F32)   # logits + bc staging

            def mish_block(psum, out_slice, li):
                """out = (psum + b_l) * (tanh(A1*p+..) + tanh(A2*p+..) + 2)

                li = layer index (0/1).  True mish needs a further /4 which is
                folded into downstream constants.
                """
                t1 = enc.tile([D, CHUNK], BF, tag="t1")
                i1 = nc.scalar.activation(t1, psum, AF.Tanh,
                                          bias=bt[:, 2 * li:2 * li + 1],
                                          scale=float(A1))
                t2 = enc.tile([D, CHUNK], BF, tag="t2")
                i2 = nc.scalar.activation(t2, psum, AF.Tanh,
                                          bias=bt[:, 2 * li + 1:2 * li + 2],
                                          scale=float(A2))
                s = enc.tile([D, CHUNK], BF, tag="s")
                nc.vector.scalar_tensor_tensor(
                    out=s, in0=t1, scalar=2.0, in1=t2, op0=ALU.add, op1=ALU.add)
                nc.vector.scalar_tensor_tensor(
                    out=out_slice, in0=psum, scalar=bv[:, li:li + 1], in1=s,
                    op0=ALU.add, op1=ALU.mult)
                return i2

            # ---- phase E: encode (Tanh table set) ----
            tanh_last = None
            for c in range(NCH):
                xsl = xq[c // 4][:, (c % 4) * CHUNK:((c % 4) + 1) * CHUNK]
                ph = psE.tile([D, CHUNK], F32, tag="pe")
                nc.tensor.matmul(ph, w1t, xsl, start=True, stop=True)
                h = enc.tile([D, CHUNK], BF, tag="h")
                mish_block(ph, h, 0)
                ph2 = psE.tile([D, CHUNK], F32, tag="pe")
                nc.tensor.matmul(ph2, w2t, h, start=True, stop=True)
                tanh_last = mish_block(
                    ph2, xdml[:, c * CHUNK:(c + 1) * CHUNK], 1)

            # ---- phase D+L: distances, sqrt, logits (Sqrt table set) ----
            sqrt_last = None
            for c in range(NCH):
                xds = xdml[:, c * CHUNK:(c + 1) * CHUNK]
                sq = dwork.tile([D, CHUNK], BF)
                nc.scalar.activation(sq, xds, AF.Square)
                px2 = psX.tile([D, CHUNK], F32, tag="x2")
                nc.tensor.matmul(px2, ones, sq, start=True, stop=True)
                x2r = dwork.tile([D, CHUNK], F32)
                nc.scalar.activation(x2r, px2, AF.Copy)

                dsb = distp.tile([D, NAT, CHUNK], BF)
                for t in range(NAT):
                    pd = psD.tile([D, CHUNK], F32, tag="pd")
                    nc.tensor.matmul(pd, m2at[:, t * 128:(t + 1) * 128], xds,
                                     start=True, stop=True)
                    nc.vector.scalar_tensor_tensor(
                        out=dsb[:, t], in0=pd, scalar=a2[:, t:t + 1], in1=x2r,
                        op0=ALU.add, op1=ALU.add)
                si = nc.scalar.activation(dsb, dsb, AF.Sqrt)
                if tanh_last is not None:
                    add_dep_helper(si.ins, tanh_last.ins, sync=False,
                                   reason="ACT table: all Tanh before Sqrt")
                sqrt_last = si

                pl = psL.tile([C, CHUNK], F32, tag="pl")
                for t in range(NAT):
                    nc.tensor.matmul(pl, wct[:, t * C:(t + 1) * C], dsb[:, t],
                                     start=(t == 0), stop=(t == NAT - 1))
                nc.scalar.activation(lsb[:, c * CHUNK:(c + 1) * CHUNK], pl,
                                     AF.Identity, bias=bc)

            # ---- phase S: log-softmax over 10 classes (Exp/Ln table set) ----
            # log_softmax without max-subtraction: |logits| < ~30 so exp is
            # safe in f32.  colsum over the 10 partitions via a ones-matmul;
            # -logdenom broadcast back with a K=1 matmul of -ones.
            for c in range(NCH):
                lsl = lsb[:, c * CHUNK:(c + 1) * CHUNK]
                e = smx.tile([C, CHUNK], BF)
                ei = nc.scalar.activation(e, lsl, AF.Exp)
                if sqrt_last is not None:
                    add_dep_helper(ei.ins, sqrt_last.ins, sync=False,
                                   reason="ACT table: all Sqrt before Exp")
                ps = psE.tile([1, CHUNK], F32, tag="pe")
                nc.tensor.matmul(ps, o10, e, start=True, stop=True)
                ld = smx.tile([1, CHUNK], F32)
                nc.scalar.activation(ld, ps, AF.Ln)
                pb = psE.tile([C, CHUNK], F32, tag="pe")
                nc.tensor.matmul(pb, n10, ld, start=True, stop=True)
                ob = smx.tile([C, CHUNK], F32)
                nc.vector.tensor_add(ob, lsl, pb)
                nc.sync.dma_start(out=out_d[:, c * CHUNK:(c + 1) * CHUNK], in_=ob)

    nc.compile()
    return nc


def _get_nc():
    if "nc" not in _CACHE:
        _CACHE["nc"] = _build()
    return _CACHE["nc"]


def _mish64(x):
    return x * np.tanh(np.log1p(np.exp(-np.abs(x))) + np.maximum(x, 0.0))


def kernel(x, mlg, W1, b1, W2, b2, Wa, Wc, bc):
    global last_result
    nc = _get_nc()

    # ---- host-side anchor precompute (x-independent, replicated) ----
    f8 = np.float64
    anch = np.tanh(Wa.astype(f8) @ mlg.astype(f8))
    anch = _mish64(anch @ W1.T.astype(f8) + b1.astype(f8))
    anch = _mish64(anch @ W2.T.astype(f8) + b2.astype(f8))          # [1024, 128]
    anch_bf = anch.astype(np.float32).astype(bf16)
    # a2 consistent with the bf16 anchors the GEMM sees (x/2 is exact in bf16)
    a2 = (anch_bf.astype(f8) ** 2).sum(1)                            # [1024]
    # device xdml is 4*mish: -2*x.a = q.(-anchors/2); x2 = sum(q^2)/16
    m2at = np.ascontiguousarray((-0.5 * anch_bf.astype(np.float32)).T
                                ).astype(bf16)                       # [128, 1024]
    a2_t = np.ascontiguousarray(a2.reshape(NAT, 128).T).astype(np.float32)

    bt = np.stack([A1 * b1 + C1, A2 * b1 + C2,
                   A1 * b2 + C1, A2 * b2 + C2], axis=1)              # [128, 4]

    common = {
        "w1t": np.ascontiguousarray(W1.T).astype(bf16),
        "w2t": np.ascontiguousarray(W2.T / 4.0).astype(bf16),
        "bt": bt.astype(np.float32),
        "bv": np.stack([b1, b2], axis=1).astype(np.float32),
        "m2at": m2at,
        "a2": a2_t,
        "wct": np.ascontiguousarray(
            Wc.T.reshape(NAT, 128, C).transpose(1, 0, 2).reshape(128, NAT * C)
        ).astype(bf16),
        "bc": bc.reshape(C, 1).astype(np.float32),
        "ones": np.full((D, D), 1.0 / 16.0, dtype=bf16),
        "o10": np.ones((C, 1), dtype=bf16),
        "n10": np.full((1, C), -1.0, dtype=np.float32),
    }

    in_maps = []
    for i in range(N_CORES):
        m = dict(common)
        m["xt"] = np.ascontiguousarray(
            x[i * RPC:(i + 1) * RPC].T).astype(bf16)
        in_maps.append(m)

    res = run_bass_kernel_spmd(
        nc, in_maps, core_ids=list(range(N_CORES)),
        trace=bool(checkenv("BASS_TRACE")),
    )
    last_result = res
    outs = [res.results[i]["out"] for i in range(N_CORES)]
    return np.concatenate([o.T for o in outs], axis=0).astype(np.float32)


# revision 13
# speedup vs baseline: 1.2216x; 1.2216x over previous
"""Trainium2 kernel for nn_AnchorMlgDml (retrieval_knn).

Pipeline per core (data-parallel over N=65536 rows, 8 cores x 8192 rows):
  x_dml = mish(mish(x @ W1.T + b1) @ W2.T + b2)            [8192, 128]
  dist  = sqrt(x2 + a2 - 2 * x_dml @ anchors.T)            [8192, 1024]
  out   = log_softmax(dist @ Wc.T + bc)                    [8192, 10]

anchors = encode_dml(tanh(Wa @ mlg)) is x-independent: computed on the host
(float64) and replicated to all cores per the sharding hint.

The toolchain's ACT tables have no Mish/Softplus, so mish is evaluated as
  mish(x) = x * (tanh(a1*x + c1) + tanh(a2*x + c2) + 2) / 4
(fitted; end-to-end error ~1.6e-3, at the bf16 floor).  The /4 is folded
into host-side constants: W2.T/4, anchors.T/2 (for -2*x.a), ones/16 (x2).

On-device layout is transposed: features on partitions, rows on the free dim.
distT [anchors(part), rows(free)] feeds the logits GEMM directly (contraction
over anchors must be on partitions).  a2 rides the DVE scalar_tensor_tensor
per-partition scalar; x2 is replicated across partitions with a ones-matmul.

ACT table sets force a phase structure (switches cost ~2.7us):
  tanh phase -> sqrt phase -> exp/ln phase, enforced with add_dep_helper.
"""

import numpy as np
import ml_dtypes

import concourse.bass as bass
import concourse.bacc as bacc
import concourse.tile as tile
import concourse.mybir as mybir
from concourse.bass_utils import run_bass_kernel_spmd, checkenv
from concourse.tile_rust import add_dep_helper

N_CORES = 8
N, D, C = 65536, 128, 10
SS = 1024                    # anchors (sample_size)
RPC = N // N_CORES           # 8192 rows per core
CHUNK = 512
NCH = RPC // CHUNK           # 16 row chunks
NAT = SS // 128              # 8 anchor tiles

# mish(x) ~= x * (sigmoid(A1*x+C1) + sigmoid(A2*x+C2)) / 2
A1, C1 = 1.89783, -0.31804
A2, C2 = 1.15145, 1.16069

BF = mybir.dt.bfloat16
F32 = mybir.dt.float32
F32R = mybir.dt.float32r
bf16 = ml_dtypes.bfloat16
AF = mybir.ActivationFunctionType
ALU = mybir.AluOpType

_CACHE = {}
last_result = None  # BassKernelResults of the most recent run (for test.py)

# Restrict the ACT table sets the load-insertion pass can pick, so every
# function resolves into one of three sets (sigmoid -> sqrt -> exp/ln) and
# the kernel pays exactly 3 ACT_TABLE_LOADs instead of ~30 (exp/ln thrash).
# Indices must be preserved (set id = position), so non-kept sets are
# emptied rather than removed.
_KEEP_SETS = {"sigmoid_and_others", "sqrt_and_others",
              "natural_log_exp_and_others"}
_orig_get_tables = bacc.get_activation_tables


def _patched_tables(arch):
    t = _orig_get_tables(arch)
    return {name: (fns if name in _KEEP_SETS else set())
            for name, fns in t.items()}


bacc.get_activation_tables = _patched_tables


def _build():
    nc = bacc.Bacc("TRN2", target_bir_lowering=False, debug=False,
                   num_devices=N_CORES)

    xt_d = nc.dram_tensor("xt", [D, RPC], BF, kind="ExternalInput").ap()
    w1t_d = nc.dram_tensor("w1t", [D, D], BF, kind="ExternalInput").ap()
    w2t_d = nc.dram_tensor("w2t", [D, D], BF, kind="ExternalInput").ap()
    # per-layer ACT biases for the two tanh args (a_i*b + c_i) and the raw
    # layer bias (the multiplicand x = psum + b)
    bt_d = nc.dram_tensor("bt", [D, 4], F32, kind="ExternalInput").ap()
    bv_d = nc.dram_tensor("bv", [D, 2], F32, kind="ExternalInput").ap()
    m2at_d = nc.dram_tensor("m2at", [D, SS], BF, kind="ExternalInput").ap()
    a2_d = nc.dram_tensor("a2", [D, NAT], F32, kind="ExternalInput").ap()
    wct_d = nc.dram_tensor("wct", [D, NAT * C], BF, kind="ExternalInput").ap()
    bc_d = nc.dram_tensor("bc", [C, 1], F32, kind="ExternalInput").ap()
    ones_d = nc.dram_tensor("ones", [D, D], BF, kind="ExternalInput").ap()
    o1010_d = nc.dram_tensor("o1010", [C, C], BF, kind="ExternalInput").ap()
    out_d = nc.dram_tensor("out", [C, RPC], F32, kind="ExternalOutput").ap()

    with tile.TileContext(nc) as tc:
        with (
            tc.tile_pool(name="consts", bufs=1) as consts,
            tc.tile_pool(name="xin", bufs=1) as xin,
            tc.tile_pool(name="enc", bufs=3) as enc,
            tc.tile_pool(name="big", bufs=1) as big,
            tc.tile_pool(name="dwork", bufs=2) as dwork,
            tc.tile_pool(name="distp", bufs=2) as distp,
            tc.tile_pool(name="smx", bufs=3) as smx,
            tc.tile_pool(name="psE", bufs=2, space="PSUM") as psE,
            tc.tile_pool(name="psD", bufs=2, space="PSUM") as psD,
            tc.tile_pool(name="psX", bufs=2, space="PSUM") as psX,
            tc.tile_pool(name="psL", bufs=2, space="PSUM") as psL,
        ):
            # ---- constants (SWDGE ring; bulk x uses the HWDGE ring).
            # Encode-critical consts first so chunk 0 can start ASAP.
            w1t = consts.tile([D, D], BF)
            nc.gpsimd.dma_start(out=w1t, in_=w1t_d)
            bt = consts.tile([D, 4], F32)
            nc.gpsimd.dma_start(out=bt, in_=bt_d)
            bv = consts.tile([D, 2], F32)
            nc.gpsimd.dma_start(out=bv, in_=bv_d)
            w2t = consts.tile([D, D], BF)
            nc.gpsimd.dma_start(out=w2t, in_=w2t_d)
            m2at = consts.tile([D, SS], BF)
            nc.gpsimd.dma_start(out=m2at, in_=m2at_d)
            a2 = consts.tile([D, NAT], F32)
            nc.gpsimd.dma_start(out=a2, in_=a2_d)
            wct = consts.tile([D, NAT * C], BF)
            nc.gpsimd.dma_start(out=wct, in_=wct_d)
            bc = consts.tile([C, 1], F32)
            nc.gpsimd.dma_start(out=bc, in_=bc_d)
            ones = consts.tile([D, D], BF)
            nc.gpsimd.dma_start(out=ones, in_=ones_d)
            o1010 = consts.tile([C, C], BF)
            nc.gpsimd.dma_start(out=o1010, in_=o1010_d)

            # ---- x input: 4 quarter DMAs so compute starts early ----
            QW = RPC // 4
            xq = []
            for i in range(4):
                t = xin.tile([D, QW], BF, tag=f"xq{i}")
                nc.sync.dma_start(out=t, in_=xt_d[:, i * QW:(i + 1) * QW])
                xq.append(t)

            xdml = big.tile([D, RPC], BF)   # 4*mish(h2) (scale folded in consts)
            lsb = big.tile([C, RPC], F32)   # logits + bc staging

            def mish_block(psum, out_slice, li):
                """out = (psum + b_l) * (sig(A1*p+..) + sig(A2*p+..))

                li = layer index (0/1).  True mish needs a further /2 which is
                folded into downstream constants.
                """
                t1 = enc.tile([D, CHUNK], BF, tag="t1")
                nc.scalar.activation(t1, psum, AF.Sigmoid,
                                     bias=bt[:, 2 * li:2 * li + 1],
                                     scale=float(A1))
                t2 = enc.tile([D, CHUNK], BF, tag="t2")
                i2 = nc.scalar.activation(t2, psum, AF.Sigmoid,
                                          bias=bt[:, 2 * li + 1:2 * li + 2],
                                          scale=float(A2))
                s = enc.tile([D, CHUNK], BF, tag="s")
                nc.vector.tensor_add(s, t1, t2)
                nc.vector.scalar_tensor_tensor(
                    out=out_slice, in0=psum, scalar=bv[:, li:li + 1], in1=s,
                    op0=ALU.add, op1=ALU.mult)
                return i2

            # ---- phase E: encode (Tanh table set) ----
            tanh_last = None
            for c in range(NCH):
                xsl = xq[c // 4][:, (c % 4) * CHUNK:((c % 4) + 1) * CHUNK]
                ph = psE.tile([D, CHUNK], F32, tag="pe")
                nc.tensor.matmul(ph, w1t, xsl, start=True, stop=True)
                h = enc.tile([D, CHUNK], BF, tag="h")
                mish_block(ph, h, 0)
                ph2 = psE.tile([D, CHUNK], F32, tag="pe")
                nc.tensor.matmul(ph2, w2t, h, start=True, stop=True)
                tanh_last = mish_block(
                    ph2, xdml[:, c * CHUNK:(c + 1) * CHUNK], 1)

            # ---- phase D+L: distances, sqrt, logits (Sqrt table set) ----
            sqrt_last = None
            for c in range(NCH):
                xds = xdml[:, c * CHUNK:(c + 1) * CHUNK]
                sq = dwork.tile([D, CHUNK], BF)
                nc.scalar.activation(sq, xds, AF.Square)
                px2 = psX.tile([D, CHUNK], F32, tag="x2")
                nc.tensor.matmul(px2, ones, sq, start=True, stop=True)
                x2r = dwork.tile([D, CHUNK], F32)
                nc.scalar.activation(x2r, px2, AF.Copy)

                dsb = distp.tile([D, NAT, CHUNK], BF)
                for t in range(NAT):
                    pd = psD.tile([D, CHUNK], F32, tag="pd")
                    nc.tensor.matmul(pd, m2at[:, t * 128:(t + 1) * 128], xds,
                                     start=True, stop=True)
                    nc.vector.scalar_tensor_tensor(
                        out=dsb[:, t], in0=pd, scalar=a2[:, t:t + 1], in1=x2r,
                        op0=ALU.add, op1=ALU.add)
                si = nc.scalar.activation(dsb, dsb, AF.Sqrt)
                if tanh_last is not None:
                    add_dep_helper(si.ins, tanh_last.ins, sync=False,
                                   reason="ACT table: all Tanh before Sqrt")
                sqrt_last = si

                pl = psL.tile([C, CHUNK], F32, tag="pl")
                for t in range(NAT):
                    nc.tensor.matmul(pl, wct[:, t * C:(t + 1) * C], dsb[:, t],
                                     start=(t == 0), stop=(t == NAT - 1))
                nc.scalar.activation(lsb[:, c * CHUNK:(c + 1) * CHUNK], pl,
                                     AF.Identity, bias=bc)

            # ---- phase S: log-softmax over 10 classes (Exp/Ln table set) ----
            # log_softmax without max-subtraction: |logits| < ~30 so exp is
            # safe in f32.  The all-ones [10,10] matmul computes the colsum
            # AND replicates it across the 10 partitions in one shot.
            for c in range(NCH):
                lsl = lsb[:, c * CHUNK:(c + 1) * CHUNK]
                e = smx.tile([C, CHUNK], BF)
                ei = nc.scalar.activation(e, lsl, AF.Exp)
                if sqrt_last is not None:
                    add_dep_helper(ei.ins, sqrt_last.ins, sync=False,
                                   reason="ACT table: all Sqrt before Exp")
                ps = psE.tile([C, CHUNK], F32, tag="pe")
                nc.tensor.matmul(ps, o1010, e, start=True, stop=True)
                ldr = smx.tile([C, CHUNK], F32)
                nc.scalar.activation(ldr, ps, AF.Ln)
                ob = smx.tile([C, CHUNK], F32)
                nc.vector.tensor_sub(ob, lsl, ldr)
                nc.sync.dma_start(out=out_d[:, c * CHUNK:(c + 1) * CHUNK], in_=ob)

    nc.compile()
    return nc


def _get_nc():
    if "nc" not in _CACHE:
        _CACHE["nc"] = _build()
    return _CACHE["nc"]


def _mish64(x):
    return x * np.tanh(np.log1p(np.exp(-np.abs(x))) + np.maximum(x, 0.0))


def kernel(x, mlg, W1, b1, W2, b2, Wa, Wc, bc):
    global last_result
    nc = _get_nc()

    # ---- host-side anchor precompute (x-independent, replicated) ----
    f8 = np.float64
    anch = np.tanh(Wa.astype(f8) @ mlg.astype(f8))
    anch = _mish64(anch @ W1.T.astype(f8) + b1.astype(f8))
    anch = _mish64(anch @ W2.T.astype(f8) + b2.astype(f8))          # [1024, 128]
    anch_bf = anch.astype(np.float32).astype(bf16)
    # a2 consistent with the bf16 anchors the GEMM sees
    a2 = (anch_bf.astype(f8) ** 2).sum(1)                            # [1024]
    # device xdml is 2*mish: -2*x.a = q.(-anchors); x2 = sum(q^2)/4
    m2at = np.ascontiguousarray((-anch_bf.astype(np.float32)).T
                                ).astype(bf16)                       # [128, 1024]
    a2_t = np.ascontiguousarray(a2.reshape(NAT, 128).T).astype(np.float32)

    bt = np.stack([A1 * b1 + C1, A2 * b1 + C2,
                   A1 * b2 + C1, A2 * b2 + C2], axis=1)              # [128, 4]

    common = {
        "w1t": np.ascontiguousarray(W1.T).astype(bf16),
        "w2t": np.ascontiguousarray(W2.T / 2.0).astype(bf16),
        "bt": bt.astype(np.float32),
        "bv": np.stack([b1, b2], axis=1).astype(np.float32),
        "m2at": m2at,
        "a2": a2_t,
        "wct": np.ascontiguousarray(
            Wc.T.reshape(NAT, 128, C).transpose(1, 0, 2).reshape(128, NAT * C)
        ).astype(bf16),
        "bc": bc.reshape(C, 1).astype(np.float32),
        "ones": np.full((D, D), 1.0 / 4.0, dtype=bf16),
        "o1010": np.ones((C, C), dtype=bf16),
    }

    in_maps = []
    for i in range(N_CORES):
        m = dict(common)
        m["xt"] = np.ascontiguousarray(
            x[i * RPC:(i + 1) * RPC].T).astype(bf16)
        in_maps.append(m)

    res = run_bass_kernel_spmd(
        nc, in_maps, core_ids=list(range(N_CORES)),
        trace=bool(checkenv("BASS_TRACE")),
    )
    last_result = res
    outs = [res.results[i]["out"] for i in range(N_CORES)]
    return np.concatenate([o.T for o in outs], axis=0).astype(np.float32)
